# revision 1
# baseline (speedup 1.0000x reference)
"""Trainium2 Bass kernel for nn_DenseExpert (soft-gated mixture of dense experts).

Math:  out[b,u] = sum_e gate[b,e] * (x[b,:] @ alpha[e]) [u] + (gate @ beta)[b,u]
Rewritten: out = y @ alpha_flat + gate @ beta, where y[b, e*I+i] = gate[b,e]*x[b,i].

Strategy (pure data parallel over batch, 8 cores):
  Per 512-row batch chunk, per core:
    1. DMA x/gate chunk in (batch-major).
    2. Scale: y_e = x * gate[:,e] on DVE/ACT (exact fp32 multiply, output
       rounded to float32r - the PE's fast 1-cycle/row fp32 mode).
    3. PE-transpose y_e tiles to i-major (bit-exact), PSUM -> SBUF copy.
    4. PE matmuls accumulate out.T[u, b] = sum_e alpha_e.T @ y_e.T
       + beta.T @ gate.T (bias folded in as a K=8 matmul).
    5. PE-transpose out.T back to batch-major (exact fp32), DMA out.
"""

from contextlib import ExitStack

import numpy as np

import concourse.bacc as bacc
import concourse.tile as tile
import concourse.mybir as mybir
from concourse.bass_utils import run_bass_kernel_spmd

F32 = mybir.dt.float32
F32R = mybir.dt.float32r

B, E, I, U = 65536, 8, 128, 128
NCORES = 8
BLOC = B // NCORES        # 8192 batch rows per core
CHUNK = 512               # batch rows per pipeline chunk
NCHUNK = BLOC // CHUNK    # 16
TPC = CHUNK // 128        # 128-row tiles per chunk


def _build():
    nc = bacc.Bacc("TRN2", target_bir_lowering=False, debug=False)

    x = nc.dram_tensor("x", [BLOC, I], F32, kind="ExternalInput").ap()
    gate = nc.dram_tensor("gate", [BLOC, E], F32, kind="ExternalInput").ap()
    alpha = nc.dram_tensor("alpha", [E, I, U], F32, kind="ExternalInput").ap()
    beta = nc.dram_tensor("beta", [E, U], F32, kind="ExternalInput").ap()
    ident = nc.dram_tensor("ident", [128, 128], F32, kind="ExternalInput").ap()
    out = nc.dram_tensor("out", [BLOC, U], F32, kind="ExternalOutput").ap()

    with tile.TileContext(nc) as tc, ExitStack() as ctx:
        const = ctx.enter_context(tc.tile_pool(name="const", bufs=1))
        xp = ctx.enter_context(tc.tile_pool(name="xp", bufs=3))
        yp = ctx.enter_context(tc.tile_pool(name="yp", bufs=4))
        ytp = ctx.enter_context(tc.tile_pool(name="ytp", bufs=2))
        op = ctx.enter_context(tc.tile_pool(name="op", bufs=2))
        ps_yt = ctx.enter_context(tc.tile_pool(name="ps_yt", bufs=3, space="PSUM"))
        ps_ot = ctx.enter_context(tc.tile_pool(name="ps_ot", bufs=2, space="PSUM"))
        ps_o = ctx.enter_context(tc.tile_pool(name="ps_o", bufs=2, space="PSUM"))

        # --- constants: alpha (i-major), beta, identity; round to f32r ---
        alpha_sb = const.tile([128, E, U], F32, tag="alpha")
        nc.sync.dma_start(alpha_sb[:], alpha.rearrange("e i u -> i e u"))
        alpha_r = const.tile([128, E, U], F32R, tag="alphar")
        nc.vector.tensor_copy(alpha_r[:], alpha_sb[:])

        beta_sb = const.tile([8, U], F32, tag="beta")
        nc.sync.dma_start(beta_sb[:], beta)
        beta_r = const.tile([8, U], F32R, tag="betar")
        nc.vector.tensor_copy(beta_r[:], beta_sb[:])

        ident_sb = const.tile([128, 128], F32, tag="ident")
        nc.sync.dma_start(ident_sb[:], ident)
        ident_r = const.tile([128, 128], F32R, tag="identr")
        nc.vector.tensor_copy(ident_r[:], ident_sb[:])
        ident_f = const.tile([128, 128], F32, tag="identf")
        nc.vector.tensor_copy(ident_f[:], ident_sb[:])

        for c in range(NCHUNK):
            row0 = c * CHUNK
            # --- loads ---
            x_sb = xp.tile([128, TPC, I], F32, tag="x")
            nc.sync.dma_start(
                x_sb[:], x[row0 : row0 + CHUNK, :].rearrange("(t p) i -> p t i", p=128)
            )
            g_sb = xp.tile([128, TPC, E], F32, tag="g")
            nc.sync.dma_start(
                g_sb[:],
                gate[row0 : row0 + CHUNK, :].rearrange("(t p) e -> p t e", p=128),
            )
            gT_sb = xp.tile([8, CHUNK], F32, tag="gT")
            nc.sync.dma_start(
                gT_sb[:], gate[row0 : row0 + CHUNK, :].rearrange("b e -> e b")
            )
            gT_r = xp.tile([8, CHUNK], F32R, tag="gTr")
            nc.vector.tensor_copy(gT_r[:], gT_sb[:])

            # --- scale + transpose per expert ---
            yT_all = ytp.tile([128, E, CHUNK], F32R, tag="yT")
            for e in range(E):
                y_e = yp.tile([128, TPC, I], F32R, tag="y")
                for t in range(TPC):
                    idx = e * TPC + t
                    if idx % 3 == 2:
                        nc.scalar.mul(
                            y_e[:, t, :], x_sb[:, t, :], g_sb[:, t, e : e + 1]
                        )
                    else:
                        nc.vector.tensor_scalar_mul(
                            y_e[:, t, :], x_sb[:, t, :], g_sb[:, t, e : e + 1]
                        )
                yT_ps = ps_yt.tile([128, CHUNK], F32R, tag="yTps")
                for t in range(TPC):
                    nc.tensor.transpose(
                        yT_ps[:, t * 128 : (t + 1) * 128], y_e[:, t, :], ident_r[:]
                    )
                if e % 2 == 0:
                    nc.scalar.copy(yT_all[:, e, :], yT_ps[:])
                else:
                    nc.vector.tensor_copy(yT_all[:, e, :], yT_ps[:])

            # --- matmuls: out.T accumulate over experts + bias ---
            oT_ps = ps_ot.tile([128, CHUNK], F32, tag="oTps")
            for e in range(E):
                nc.tensor.matmul(
                    oT_ps[:],
                    alpha_r[:, e, :],
                    yT_all[:, e, :],
                    start=(e == 0),
                    stop=False,
                )
            nc.tensor.matmul(oT_ps[:], beta_r[:], gT_r[:], start=False, stop=True)

            oT_sb = op.tile([128, CHUNK], F32, tag="oT")
            nc.scalar.copy(oT_sb[:], oT_ps[:])

            # --- transpose back to batch-major and store ---
            o_sb = op.tile([128, TPC, U], F32, tag="o")
            for t in range(TPC):
                o_ps = ps_o.tile([128, 128], F32, tag="ops")
                nc.tensor.transpose(
                    o_ps[:], oT_sb[:, t * 128 : (t + 1) * 128], ident_f[:]
                )
                nc.vector.tensor_copy(o_sb[:, t, :], o_ps[:])
            nc.sync.dma_start(
                out[row0 : row0 + CHUNK, :].rearrange("(t p) u -> p t u", p=128),
                o_sb[:],
            )

    nc.compile()
    return nc


_NC_CACHE = None


def kernel(x, gate_perc, alpha, beta):
    global _NC_CACHE
    x = np.ascontiguousarray(np.asarray(x, dtype=np.float32))
    gate_perc = np.ascontiguousarray(np.asarray(gate_perc, dtype=np.float32))
    alpha = np.ascontiguousarray(np.asarray(alpha, dtype=np.float32))
    beta = np.ascontiguousarray(np.asarray(beta, dtype=np.float32))

    if _NC_CACHE is None:
        _NC_CACHE = _build()
    nc = _NC_CACHE

    ident = np.eye(128, dtype=np.float32)
    in_maps = []
    for c in range(NCORES):
        sl = slice(c * BLOC, (c + 1) * BLOC)
        in_maps.append(
            {
                "x": x[sl],
                "gate": gate_perc[sl],
                "alpha": alpha,
                "beta": beta,
                "ident": ident,
            }
        )
    res = run_bass_kernel_spmd(nc, in_maps, list(range(NCORES))).results
    return np.concatenate([res[c]["out"] for c in range(NCORES)], axis=0)


if __name__ == "__main__":
    rng = np.random.default_rng(0)
    x = rng.standard_normal((B, I)).astype(np.float32)
    g = rng.random((B, E)).astype(np.float32)
    g /= g.sum(-1, keepdims=True)
    al = (rng.standard_normal((E, I, U)) * 0.05).astype(np.float32)
    be = (rng.standard_normal((E, U)) * 0.05).astype(np.float32)
    got = kernel(x, g, al, be)
    ref = np.einsum("bi,eio->beo", x, al, optimize=True)
    ref = np.einsum("beo,be->bo", ref, g) + g @ be
    err = np.abs(got - ref)
    print("max abs err", err.max(), "rel", err.max() / np.abs(ref).max())


# revision 5
# speedup vs baseline: 1.3797x; 1.3797x over previous
"""Trainium2 Bass kernel for nn_DenseExpert (soft-gated mixture of dense experts).

Math:  out[b,u] = sum_e gate[b,e] * (x[b,:] @ alpha[e]) [u] + (gate @ beta)[b,u]
Rewritten: out = y @ alpha_flat + gate @ beta, where y[b, e*I+i] = gate[b,e]*x[b,i].

Strategy (pure data parallel over batch, 8 cores). Per 512-row chunk per core:
  1. DMA x/gate chunk (batch-major).
  2. Scale: y_e = x * gate[:,e] on DVE/ACT, exact fp32 (DVE 2x mode).
  3. PE-transpose y_e tiles to i-major (bit-exact fp32); the PSUM->SBUF copy
     rounds to float32r (the PE's fast 1-cycle/row fp32 matmul mode).
  4. PE matmuls accumulate out.T[u,b] = sum_e alpha_e.T @ y_e.T  plus the
     bias as K=8 matmuls beta.T @ gate.T (gate.T via one PE transpose).
  5. out.T copied to SBUF and DMA'd to DRAM in [U, B] layout; the host does
     the final cheap transpose when assembling the full result.
"""

from contextlib import ExitStack

import numpy as np

import concourse.bacc as bacc
import concourse.tile as tile
import concourse.mybir as mybir
from concourse.bass_utils import run_bass_kernel_spmd

F32 = mybir.dt.float32
F32R = mybir.dt.float32r

B, E, I, U = 65536, 8, 128, 128
NCORES = 8
BLOC = B // NCORES        # 8192 batch rows per core
CHUNK = 512               # batch rows per pipeline chunk
NCHUNK = BLOC // CHUNK    # 16
TPC = CHUNK // 128        # 128-row tiles per chunk


def _build():
    nc = bacc.Bacc("TRN2", target_bir_lowering=False, debug=False)

    x = nc.dram_tensor("x", [BLOC, I], F32, kind="ExternalInput").ap()
    gate = nc.dram_tensor("gate", [BLOC, E], F32, kind="ExternalInput").ap()
    alpha = nc.dram_tensor("alpha", [E, I, U], F32, kind="ExternalInput").ap()
    beta = nc.dram_tensor("beta", [E, U], F32, kind="ExternalInput").ap()
    ident = nc.dram_tensor("ident", [128, 128], F32, kind="ExternalInput").ap()
    # output stays feature-major on HW; host transposes when assembling
    outT = nc.dram_tensor("outT", [U, BLOC], F32, kind="ExternalOutput").ap()

    with tile.TileContext(nc) as tc, ExitStack() as ctx:
        const = ctx.enter_context(tc.tile_pool(name="const", bufs=1))
        xp = ctx.enter_context(tc.tile_pool(name="xp", bufs=4))
        yp = ctx.enter_context(tc.tile_pool(name="yp", bufs=6))
        ytp = ctx.enter_context(tc.tile_pool(name="ytp", bufs=2))
        op = ctx.enter_context(tc.tile_pool(name="op", bufs=3))
        gp = ctx.enter_context(tc.tile_pool(name="gp", bufs=3))
        ps_yt = ctx.enter_context(tc.tile_pool(name="ps_yt", bufs=3, space="PSUM"))
        ps_ot = ctx.enter_context(tc.tile_pool(name="ps_ot", bufs=2, space="PSUM"))
        ps_gt = ctx.enter_context(tc.tile_pool(name="ps_gt", bufs=2, space="PSUM"))

        # --- constants ---
        alpha_sb = const.tile([128, E, U], F32, tag="alpha")
        nc.sync.dma_start(alpha_sb[:], alpha.rearrange("e i u -> i e u"))
        alpha_r = const.tile([128, E, U], F32R, tag="alphar")
        nc.vector.tensor_copy(alpha_r[:], alpha_sb[:])

        beta_sb = const.tile([8, U], F32, tag="beta")
        nc.sync.dma_start(beta_sb[:], beta)
        beta_r = const.tile([8, U], F32R, tag="betar")
        nc.vector.tensor_copy(beta_r[:], beta_sb[:])

        ident_sb = const.tile([128, 128], F32, tag="ident")
        nc.sync.dma_start(ident_sb[:], ident)
        ident_f = const.tile([128, 128], F32, tag="identf")
        nc.vector.tensor_copy(ident_f[:], ident_sb[:])

        for c in range(NCHUNK):
            row0 = c * CHUNK
            # --- loads ---
            x_sb = xp.tile([128, TPC, I], F32, tag="x")
            nc.sync.dma_start(
                x_sb[:], x[row0 : row0 + CHUNK, :].rearrange("(t p) i -> p t i", p=128)
            )
            g_sb = xp.tile([128, TPC, E], F32, tag="g")
            nc.sync.dma_start(
                g_sb[:],
                gate[row0 : row0 + CHUNK, :].rearrange("(t p) e -> p t e", p=128),
            )

            # gate.T via PE transposes (one per 128-row tile), all at
            # partition base 0; rounded to f32r on the PSUM->SBUF copy.
            gT_ps = ps_gt.tile([E, TPC, 128], F32, tag="gTps")
            for t in range(TPC):
                nc.tensor.transpose(gT_ps[:, t, :], g_sb[:, t, :], ident_f[:])
            gT_r = gp.tile([E, TPC, 128], F32R, tag="gTr")
            nc.vector.tensor_copy(gT_r[:], gT_ps[:])

            # --- scale + transpose per expert ---
            yT_all = ytp.tile([128, E, CHUNK], F32R, tag="yT")
            for e in range(E):
                y_e = yp.tile([128, TPC, I], F32, tag="y")
                for t in range(TPC):
                    idx = e * TPC + t
                    if idx % 3 == 2:
                        nc.scalar.mul(
                            y_e[:, t, :], x_sb[:, t, :], g_sb[:, t, e : e + 1]
                        )
                    else:
                        nc.vector.tensor_scalar_mul(
                            y_e[:, t, :], x_sb[:, t, :], g_sb[:, t, e : e + 1]
                        )
                yT_ps = ps_yt.tile([128, CHUNK], F32, tag="yTps")
                for t in range(TPC):
                    nc.tensor.transpose(
                        yT_ps[:, t * 128 : (t + 1) * 128], y_e[:, t, :], ident_f[:]
                    )
                # PSUM->SBUF copy doubles as the f32r rounding
                if e % 2 == 0:
                    nc.scalar.copy(yT_all[:, e, :], yT_ps[:])
                else:
                    nc.vector.tensor_copy(yT_all[:, e, :], yT_ps[:])

            # --- matmuls: out.T accumulate over experts + bias ---
            oT_ps = ps_ot.tile([128, CHUNK], F32, tag="oTps")
            for e in range(E):
                nc.tensor.matmul(
                    oT_ps[:],
                    alpha_r[:, e, :],
                    yT_all[:, e, :],
                    start=(e == 0),
                    stop=False,
                )
            for t in range(TPC):
                nc.tensor.matmul(
                    oT_ps[:, t * 128 : (t + 1) * 128],
                    beta_r[:],
                    gT_r[:, t, :],
                    start=False,
                    stop=(t == TPC - 1),
                )

            oT_sb = op.tile([128, CHUNK], F32, tag="oT")
            nc.scalar.copy(oT_sb[:], oT_ps[:])
            nc.sync.dma_start(outT[:, row0 : row0 + CHUNK], oT_sb[:])

    nc.compile()
    return nc


_NC_CACHE = None


def kernel(x, gate_perc, alpha, beta):
    global _NC_CACHE
    x = np.ascontiguousarray(np.asarray(x, dtype=np.float32))
    gate_perc = np.ascontiguousarray(np.asarray(gate_perc, dtype=np.float32))
    alpha = np.ascontiguousarray(np.asarray(alpha, dtype=np.float32))
    beta = np.ascontiguousarray(np.asarray(beta, dtype=np.float32))

    if _NC_CACHE is None:
        _NC_CACHE = _build()
    nc = _NC_CACHE

    ident = np.eye(128, dtype=np.float32)
    in_maps = []
    for c in range(NCORES):
        sl = slice(c * BLOC, (c + 1) * BLOC)
        in_maps.append(
            {
                "x": x[sl],
                "gate": gate_perc[sl],
                "alpha": alpha,
                "beta": beta,
                "ident": ident,
            }
        )
    res = run_bass_kernel_spmd(nc, in_maps, list(range(NCORES))).results
    # per-core outputs are [U, BLOC]; assemble and transpose on host
    full_T = np.concatenate([res[c]["outT"] for c in range(NCORES)], axis=1)
    return np.ascontiguousarray(full_T.T)


if __name__ == "__main__":
    rng = np.random.default_rng(0)
    x = rng.standard_normal((B, I)).astype(np.float32)
    g = rng.random((B, E)).astype(np.float32)
    g /= g.sum(-1, keepdims=True)
    al = (rng.standard_normal((E, I, U)) * 0.05).astype(np.float32)
    be = (rng.standard_normal((E, U)) * 0.05).astype(np.float32)
    got = kernel(x, g, al, be)
    ref = np.einsum("bi,eio->beo", x, al, optimize=True)
    ref = np.einsum("beo,be->bo", ref, g) + g @ be
    err = np.abs(got - ref)
    print("max abs err", err.max(), "rel", err.max() / np.abs(ref).max())
